# revision 1
# baseline (speedup 1.0000x reference)
"""Bahdanau attention kernel for Trainium2 (8 NeuronCores, data-parallel over batch).

Reference computation (per batch row b):
    pq      = query @ Wq.T                       # (B, AD)
    hidden  = tanh(pq[:, None, :] + processed_memory)   # (B, T, AD)
    e       = einsum('btd,d->bt', hidden, v)     # (B, T)
    e       = where(mask, -1e30, e)
    out     = softmax(e, axis=1)

Device strategy (per core, 8 batches):
  * processed_memory is host-transposed to [b, AD, T] so AD sits on SBUF
    partitions.  The per-d "+pq" add then folds into the ScalarE tanh as a
    per-partition activation bias (free), and the v-weighted reduction over d
    becomes TensorE matmuls with a [128,1] stationary v column (M=1, free up
    to 512) accumulating in PSUM.
  * Energies strips [1, 2048] leave PSUM via a VectorE copy, then tiny
    SBUF->SBUF DMAs relayout them into an [8, T] tile (one batch per
    partition) where the masked softmax runs along the free dimension:
    exp on ScalarE, mask-multiply + row-sum fused in one
    tensor_tensor_reduce, reciprocal + scale on VectorE.
  * mask is applied multiplicatively: softmax(where(m,-1e30,e)) ==
    exp(e)*(1-m) / sum(exp(e)*(1-m)) exactly (exp(-1e30) underflows to 0,
    and |e| <= sum|v| ~ 13 so exp(e) cannot overflow in fp32).
"""

import sys

if "/opt/trn_rl_repo" not in sys.path:
    sys.path.insert(0, "/opt/trn_rl_repo")

import numpy as np

import concourse.bacc as bacc
import concourse.bass as bass
import concourse.tile as tile
from concourse import mybir
from concourse.bass_utils import run_bass_kernel_spmd

B, T, QD, AD = 64, 4096, 1024, 256
NCORES = 8
BLOC = B // NCORES  # batches per core
KB = QD // 128      # k-blocks for the pq matmul
DB = AD // 128      # d-blocks (partition blocks of AD)
F32 = mybir.dt.float32
F16 = mybir.dt.float16
U8 = mybir.dt.uint8


def build_nc() -> bass.Bass:
    # Bacc (not plain Bass): its nop/event-semaphore lowering passes are what
    # let Tile-scheduled instructions carry multiple semaphore waits.
    nc = bacc.Bacc(None, target_bir_lowering=False)

    # fp16: halves the dominant HBM stream; pm ~ N(0,1) so fp16 quantization
    # (10 mantissa bits) costs ~2e-4 rel err on the softmax output
    pm_t = nc.declare_dram_parameter("pm_t", [BLOC, AD, T], F16, isOutput=False)
    # qT[p, kb*BLOC + b] = query[b, kb*128 + p]  (host-packed, partition-major)
    qT = nc.declare_dram_parameter("qT", [128, KB * BLOC], F32, isOutput=False)
    msk = nc.declare_dram_parameter("mask", [BLOC, T], U8, isOutput=False)
    WqT = nc.declare_dram_parameter("WqT", [QD, AD], F32, isOutput=False)
    v_r = nc.declare_dram_parameter("v_r", [128, DB], F32, isOutput=False)
    # block-indicator matrices for the softmax cross-partition matmuls:
    # sel16[p, b] = 1.0 iff p // 16 == b ; sel16T is its transpose
    sel16_d = nc.declare_dram_parameter("sel16", [128, B // NCORES], F32, isOutput=False)
    sel16T_d = nc.declare_dram_parameter("sel16T", [B // NCORES, 128], F32, isOutput=False)
    out = nc.declare_dram_parameter("out", [BLOC, T], F32, isOutput=True)

    Tanh = mybir.ActivationFunctionType.Tanh
    Exp = mybir.ActivationFunctionType.Exp
    mult = mybir.AluOpType.mult
    add = mybir.AluOpType.add

    HT = 2048          # energies strip length (4 PSUM banks)
    NMM = HT // 512    # matmuls per strip per d-block
    PB = 16            # partitions per batch in the softmax layout
    PF = T // PB       # 256 free elements per partition

    with tile.TileContext(nc) as tc:
        with (
            tc.tile_pool(name="singles", bufs=1) as singles,
            tc.tile_pool(name="pm", bufs=8) as pm_pool,
            tc.tile_pool(name="hid", bufs=6) as hid_pool,
            tc.tile_pool(name="estrip", bufs=4) as estrip_pool,
            tc.tile_pool(name="epsum", bufs=2, space="PSUM") as epsum_pool,
        ):
            # ---- constant loads (wq/qt first: they gate pq -> first tanh) ----
            wq_sb = singles.tile([128, KB, AD], F32)
            nc.sync.dma_start(
                out=wq_sb, in_=WqT[:, :].rearrange("(kb p) d -> p kb d", p=128)
            )
            qt_sb = singles.tile([128, KB, BLOC], F32)
            nc.sync.dma_start(
                out=qt_sb, in_=qT[:, :].rearrange("p (kb b) -> p kb b", b=BLOC)
            )
            v_sb = singles.tile([128, DB], F32)
            nc.sync.dma_start(out=v_sb, in_=v_r[:, :])
            # fp16 copy of v for the energies matmuls: fp32 matmuls run as
            # two PE passes at ~4x the cost; tanh outputs are in [-1,1] and
            # v is small, so fp16 (10 mantissa bits) costs ~3e-4 rel err.
            v16_sb = singles.tile([128, DB], F16)
            nc.vector.tensor_copy(out=v16_sb, in_=v_sb)

            # ---- pq = Wq @ query.T, laid out [d % 128, dblk, b] ----
            pq_sb = singles.tile([128, DB, BLOC], F32)
            for d in range(DB):
                ppq = epsum_pool.tile([128, BLOC], F32, tag="ep")
                for k in range(KB):
                    nc.tensor.matmul(
                        ppq,
                        lhsT=wq_sb[:, k, d * 128 : (d + 1) * 128],
                        rhs=qt_sb[:, k, :],
                        start=(k == 0),
                        stop=(k == KB - 1),
                    )
                nc.scalar.copy(pq_sb[:, d, :], ppq)

            e2_sb = singles.tile([128, PF], F32)
            work2 = singles.tile([128, PF], F32)
            colsum = singles.tile([128, 1], F32)
            rinv_sb = singles.tile([BLOC, 1], F32)

            # ---- main loop: tanh + v-reduction ----
            for b in range(BLOC):
                hid = []
                for d in range(DB):
                    pm_sb = pm_pool.tile([128, T], F16)
                    nc.sync.dma_start(
                        out=pm_sb, in_=pm_t[b, d * 128 : (d + 1) * 128, :]
                    )
                    h = hid_pool.tile([128, T], F16)
                    nc.scalar.activation(
                        out=h,
                        in_=pm_sb,
                        func=Tanh,
                        bias=pq_sb[:, d, b : b + 1],
                        scale=1.0,
                    )
                    hid.append(h)
                for half in range(T // HT):
                    ep = epsum_pool.tile([1, HT], F32, tag="ep")
                    for c in range(NMM):
                        lo = half * HT + c * 512
                        nc.tensor.matmul(
                            ep[:, c * 512 : (c + 1) * 512],
                            lhsT=v16_sb[:, 0:1],
                            rhs=hid[0][:, lo : lo + 512],
                            start=True,
                            stop=False,
                        )
                        nc.tensor.matmul(
                            ep[:, c * 512 : (c + 1) * 512],
                            lhsT=v16_sb[:, 1:2],
                            rhs=hid[1][:, lo : lo + 512],
                            start=False,
                            stop=True,
                        )
                    es = estrip_pool.tile([1, HT], F32)
                    nc.vector.tensor_copy(out=es, in_=ep)
                    p0 = b * PB + half * (HT // PF)
                    nc.gpsimd.dma_start(
                        out=e2_sb[p0 : p0 + HT // PF, :], in_=es
                    )

            # ---- softmax-side constants ----
            # energies layout for the post pass: partition p = b*PB + q holds
            # t in [ (p%PB)*PF, ... ) of batch b = p//PB -> all 128 partitions
            # work during the softmax instead of 8.
            mask2_sb = singles.tile([128, PF], U8)
            nc.sync.dma_start(
                out=mask2_sb, in_=msk[:, :].rearrange("b (q f) -> (b q) f", f=PF)
            )
            maskz2_sb = singles.tile([128, PF], F32)
            nc.vector.tensor_scalar(
                out=maskz2_sb,
                in0=mask2_sb,
                scalar1=-1.0,
                scalar2=1.0,
                op0=mult,
                op1=add,
            )
            sel16 = singles.tile([128, BLOC], F32)
            nc.sync.dma_start(out=sel16, in_=sel16_d[:, :])
            sel16T = singles.tile([BLOC, 128], F32)
            nc.sync.dma_start(out=sel16T, in_=sel16T_d[:, :])


            # ---- masked softmax, all 128 partitions busy ----
            nc.scalar.activation(out=work2, in_=e2_sb, func=Exp)
            # (tensor_tensor_reduce is a custom ant-dve ucode op that faults
            # on this runtime — use the two standard ops instead)
            nc.vector.tensor_mul(work2, work2, maskz2_sb)
            nc.vector.reduce_sum(out=colsum, in_=work2, axis=mybir.AxisListType.X)
            # per-batch row sums: rowsum[b] = sum_p sel16[p, b] * colsum[p]
            psum_rs = epsum_pool.tile([BLOC, 1], F32, tag="ep")
            nc.tensor.matmul(psum_rs, lhsT=sel16, rhs=colsum, start=True, stop=True)
            nc.vector.reciprocal(out=rinv_sb, in_=psum_rs)
            # broadcast 1/rowsum back to the 16 partitions of each batch
            psum_ri = epsum_pool.tile([128, 1], F32, tag="ep")
            nc.tensor.matmul(psum_ri, lhsT=sel16T, rhs=rinv_sb, start=True, stop=True)
            nc.vector.tensor_scalar_mul(out=work2, in0=work2, scalar1=psum_ri)
            nc.sync.dma_start(
                out=out[:, :].rearrange("b (q f) -> (b q) f", f=PF), in_=work2
            )

    # Run the Bacc lowering passes (move_matmul_waits_to_ldweights,
    # generate_event_semaphores, alloc_regs, ...) — run_bass_via_pjrt takes
    # the module as-is and walrus rejects unlowered multi-wait instructions.
    nc.finalize()
    return nc


_CACHE: dict = {}


def _get_nc() -> bass.Bass:
    if "nc" not in _CACHE:
        _CACHE["nc"] = build_nc()
    return _CACHE["nc"]


def make_in_maps(query, processed_memory, mask, Wq, v):
    query = np.ascontiguousarray(np.asarray(query, dtype=np.float32))
    pm = np.asarray(processed_memory, dtype=np.float32)
    mask_u8 = np.asarray(mask).astype(np.uint8)
    Wq = np.asarray(Wq, dtype=np.float32)
    v = np.asarray(v, dtype=np.float32)

    WqT = np.ascontiguousarray(Wq.T)                  # (QD, AD)
    v_r = np.ascontiguousarray(v.reshape(DB, 128).T)  # (128, DB)
    sel16 = np.zeros((128, BLOC), dtype=np.float32)
    for b in range(BLOC):
        sel16[b * 16 : (b + 1) * 16, b] = 1.0
    sel16T = np.ascontiguousarray(sel16.T)

    in_maps = []
    for i in range(NCORES):
        sl = slice(i * BLOC, (i + 1) * BLOC)
        in_maps.append(
            {
                "pm_t": np.ascontiguousarray(
                    pm[sl].transpose(0, 2, 1).astype(np.float16)
                ),
                "qT": np.ascontiguousarray(
                    query[sl]
                    .T.reshape(KB, 128, BLOC)
                    .transpose(1, 0, 2)
                    .reshape(128, KB * BLOC)
                ),
                "mask": np.ascontiguousarray(mask_u8[sl]),
                "WqT": WqT,
                "v_r": v_r,
                "sel16": sel16,
                "sel16T": sel16T,
            }
        )
    return in_maps


def run_spmd(in_maps, **kwargs):
    return run_bass_kernel_spmd(_get_nc(), in_maps, list(range(NCORES)), **kwargs)


def kernel(query, processed_memory, mask, Wq, v) -> np.ndarray:
    in_maps = make_in_maps(query, processed_memory, mask, Wq, v)
    res = run_spmd(in_maps)
    return np.concatenate(
        [res.results[i]["out"] for i in range(NCORES)], axis=0
    ).astype(np.float32)



# revision 3
# speedup vs baseline: 1.3686x; 1.3686x over previous
"""Bahdanau attention kernel for Trainium2 (8 NeuronCores, data-parallel over batch).

Reference computation (per batch row b):
    pq      = query @ Wq.T                       # (B, AD)
    hidden  = tanh(pq[:, None, :] + processed_memory)   # (B, T, AD)
    e       = einsum('btd,d->bt', hidden, v)     # (B, T)
    e       = where(mask, -1e30, e)
    out     = softmax(e, axis=1)

Key observation: ~50% of positions have mask=True, and for those the reference
output is *exactly* 0.0 (exp(-1e30 - max) underflows).  So the host gathers
only the unmasked positions per batch (n_b <= 2126 for this input family),
pads each batch to a common static length P, and the device only processes
the compacted stream - halving HBM traffic, tanh work and matmul work.  The
host scatters results back and fills masked positions with exact zeros.

Device strategy (per core, 8 batches, compacted to [*, P]):
  * pm is host-gathered/transposed to [b, d-block, 128, P] fp16 so AD sits on
    SBUF partitions.  The "+pq" add folds into the ScalarE tanh as a
    per-partition activation bias, and the v-weighted reduction over d is
    TensorE matmuls with a [128,1] fp16 stationary v column.
  * Energy strips for a group of 4 batches are col-tiled to PSUM partitions
    0/32/64/96 of one [128, P] strip tile (tile_position=(0, 32j) is implied
    by the output base partition).  This makes the PSUM->SBUF evacuation a
    [128, 512]-shaped DVE copy (all lanes busy) instead of the [1, 2048]
    single-lane copy that cost 2.3us each in the previous version.
  * Strips are relaid to a [128, P/16] softmax tile (16 partitions per batch)
    via SWDGE SBUF->SBUF DMAs that *accumulate* onto a preloaded additive
    mask (0 for valid, -50 for padding), so masking costs no extra engine op.
  * Softmax: ScalarE exp with fused accum_out row sums, per-batch sums via a
    [128,8] selector matmul, VectorE reciprocal, broadcast back via a second
    selector matmul, and one scale+store.
  * Padded positions contribute exp(e-50) <= 3e-17 to the denominator
    (relative ~1e-19) and are discarded by the host scatter.
"""

import sys

if "/opt/trn_rl_repo" not in sys.path:
    sys.path.insert(0, "/opt/trn_rl_repo")

import numpy as np

import concourse.bacc as bacc
import concourse.bass as bass
import concourse.tile as tile
from concourse import mybir
from concourse.bass_utils import run_bass_kernel_spmd

B, T, QD, AD = 64, 4096, 1024, 256
NCORES = 8
BLOC = B // NCORES  # batches per core
KB = QD // 128      # k-blocks for the pq matmul
DB = AD // 128      # d-blocks (partition blocks of AD)
F32 = mybir.dt.float32
F16 = mybir.dt.float16
U8 = mybir.dt.uint8

P_DEFAULT = 2176    # compacted positions per batch (multiple of 16, >= max n_b)


def build_nc(P: int) -> bass.Bass:
    assert P % 16 == 0 and P <= 2560
    PF = P // 16            # free elements per partition in the softmax layout
    # matmul chunks: PSUM bank limit is 512 fp32 columns per matmul
    chunks = [(c, min(c + 512, P)) for c in range(0, P, 512)]

    nc = bacc.Bacc(None, target_bir_lowering=False)

    pm_c = nc.declare_dram_parameter("pm_c", [BLOC, DB, 128, P], F16, isOutput=False)
    # qT16[p, kb*BLOC + b] = query[b, kb*128 + p]  (host-packed, partition-major)
    qT = nc.declare_dram_parameter("qT", [128, KB * BLOC], F16, isOutput=False)
    WqT = nc.declare_dram_parameter("WqT", [QD, AD], F16, isOutput=False)
    v_r = nc.declare_dram_parameter("v_r", [128, DB], F16, isOutput=False)
    mko = nc.declare_dram_parameter("mko", [BLOC, P], U8, isOutput=False)  # 1=valid
    sel16_d = nc.declare_dram_parameter("sel16", [128, BLOC], F32, isOutput=False)
    sel16T_d = nc.declare_dram_parameter("sel16T", [BLOC, 128], F32, isOutput=False)
    out = nc.declare_dram_parameter("out", [BLOC, P], F32, isOutput=True)

    Tanh = mybir.ActivationFunctionType.Tanh
    Exp = mybir.ActivationFunctionType.Exp
    mult = mybir.AluOpType.mult
    add = mybir.AluOpType.add

    with tile.TileContext(nc) as tc:
        with (
            tc.tile_pool(name="singles", bufs=1) as singles,
            tc.tile_pool(name="pm", bufs=4) as pm_pool,
            tc.tile_pool(name="hid", bufs=6) as hid_pool,
            tc.tile_pool(name="es", bufs=2) as es_pool,
            tc.tile_pool(name="ps", bufs=1, space="PSUM") as ps_pool,
        ):
            # ---- tiny dummy tanh issued first: pulls the ~2.7us
            # ACT_TABLE_LOAD (exp_and_others covers tanh+exp) off the
            # critical path while DMAs/pq run ----
            dummy = singles.tile([128, 1], F32)
            nc.gpsimd.memset(dummy, 0.0)
            dummy2 = singles.tile([128, 1], F32)
            nc.scalar.activation(out=dummy2, in_=dummy, func=Tanh)

            # ---- constant loads (wq/qt first: they gate pq -> first tanh) ----
            wq_sb = singles.tile([128, KB, AD], F16)
            for db in range(DB):
                nc.sync.dma_start(
                    out=wq_sb[:, :, db * 128 : (db + 1) * 128],
                    in_=WqT[:, db * 128 : (db + 1) * 128].rearrange(
                        "(kb p) d -> p kb d", p=128
                    ),
                )
            qt_sb = singles.tile([128, KB, BLOC], F16)
            nc.sync.dma_start(
                out=qt_sb, in_=qT[:, :].rearrange("p (kb b) -> p kb b", b=BLOC)
            )
            v16 = singles.tile([128, DB], F16)
            nc.sync.dma_start(out=v16, in_=v_r[:, :])

            # ---- pq = Wq @ query.T, laid out [d % 128, dblk, b] (fp16, 1-pass) ----
            pq_sb = singles.tile([128, DB, BLOC], F32)
            for db in range(DB):
                ppq = ps_pool.tile([128, BLOC], F32, tag="pq", bufs=2)
                for k in range(KB):
                    nc.tensor.matmul(
                        ppq,
                        lhsT=wq_sb[:, k, db * 128 : (db + 1) * 128],
                        rhs=qt_sb[:, k, :],
                        start=(k == 0),
                        stop=(k == KB - 1),
                    )
                nc.vector.tensor_copy(out=pq_sb[:, db, :], in_=ppq)

            # ---- softmax-side constants ----
            mask_u8 = singles.tile([128, PF], U8)
            nc.sync.dma_start(
                out=mask_u8, in_=mko[:, :].rearrange("b (q f) -> (b q) f", f=PF)
            )
            # additive mask: 0.0 where valid, -50.0 where padding
            maskadd = singles.tile([128, PF], F32)
            nc.vector.tensor_scalar(
                out=maskadd,
                in0=mask_u8,
                scalar1=50.0,
                scalar2=-50.0,
                op0=mult,
                op1=add,
            )
            sel16 = singles.tile([128, BLOC], F32)
            nc.sync.dma_start(out=sel16, in_=sel16_d[:, :])
            sel16T = singles.tile([BLOC, 128], F32)
            nc.sync.dma_start(out=sel16T, in_=sel16T_d[:, :])

            # energies accumulate into e2 on top of maskadd via DMA accum
            e2 = singles.tile([128, PF], F32)
            nc.vector.tensor_copy(out=e2, in_=maskadd)

            work2 = singles.tile([128, PF], F32)
            colsum = singles.tile([128, 1], F32)
            rinv = singles.tile([BLOC, 1], F32)

            # ---- main loop ----
            ep = es = None
            for b in range(BLOC):
                g, j = b // 4, b % 4
                if j == 0:
                    # group strip: 4 batches at PSUM partitions 0/32/64/96
                    ep = ps_pool.tile([128, P], F32, tag="strip", bufs=1)
                    es = es_pool.tile([128, P], F32)
                hids = []
                for db in range(DB):
                    pm_sb = pm_pool.tile([128, P], F16)
                    nc.sync.dma_start(out=pm_sb, in_=pm_c[b, db, :, :])
                    h = hid_pool.tile([128, P], F16)
                    nc.scalar.activation(
                        out=h,
                        in_=pm_sb,
                        func=Tanh,
                        bias=pq_sb[:, db, b : b + 1],
                        scale=1.0,
                    )
                    hids.append(h)
                row = ep[32 * j : 32 * j + 1, :]
                for c0, c1 in chunks:
                    nc.tensor.matmul(
                        row[:, c0:c1],
                        lhsT=v16[:, 0:1],
                        rhs=hids[0][:, c0:c1],
                        start=True,
                        stop=False,
                        tile_position=(0, 32 * j),
                    )
                    nc.tensor.matmul(
                        row[:, c0:c1],
                        lhsT=v16[:, 1:2],
                        rhs=hids[1][:, c0:c1],
                        start=False,
                        stop=True,
                        tile_position=(0, 32 * j),
                    )
                if j == 3:
                    # evacuate the group strip: full-width DVE copies
                    for c0, c1 in chunks:
                        nc.vector.tensor_copy(out=es[:, c0:c1], in_=ep[:, c0:c1])
                    # relayout each batch row into the [128, PF] softmax tile,
                    # accumulating onto the preloaded additive mask
                    for bb in range(g * 4, g * 4 + 4):
                        jj = bb % 4
                        nc.gpsimd.dma_start(
                            out=e2[bb * 16 : (bb + 1) * 16, :],
                            in_=es[32 * jj : 32 * jj + 1, :],
                            accum_op=add,
                        )
                    # masked exp for this half, row sums fused into accum_out
                    lo, hi = g * 64, g * 64 + 64
                    nc.scalar.activation(
                        out=work2[lo:hi, :],
                        in_=e2[lo:hi, :],
                        func=Exp,
                        accum_out=colsum[lo:hi, :],
                    )

            # ---- finish softmax ----
            psum_rs = ps_pool.tile([BLOC, 1], F32, tag="red", bufs=1)
            nc.tensor.matmul(psum_rs, lhsT=sel16, rhs=colsum, start=True, stop=True)
            nc.vector.reciprocal(out=rinv, in_=psum_rs)
            psum_ri = ps_pool.tile([128, 1], F32, tag="red", bufs=1)
            nc.tensor.matmul(psum_ri, lhsT=sel16T, rhs=rinv, start=True, stop=True)
            nc.vector.tensor_scalar_mul(out=e2, in0=work2, scalar1=psum_ri)
            nc.sync.dma_start(
                out=out[:, :].rearrange("b (q f) -> (b q) f", f=PF), in_=e2
            )

    nc.finalize()
    return nc


_CACHE: dict = {}


def _get_nc(P: int) -> bass.Bass:
    if P not in _CACHE:
        _CACHE[P] = build_nc(P)
    return _CACHE[P]


def prep(query, processed_memory, mask, Wq, v):
    """Host-side shard + compact.  Returns (P, in_maps, scatter_info)."""
    query = np.asarray(query, dtype=np.float32)
    pm = np.asarray(processed_memory, dtype=np.float32)
    mask_b = np.asarray(mask).astype(bool)
    Wq = np.asarray(Wq, dtype=np.float32)
    v = np.asarray(v, dtype=np.float32)

    idxs = [np.nonzero(~mask_b[b])[0] for b in range(B)]
    nmax = max((len(ix) for ix in idxs), default=0)
    P = max(P_DEFAULT, -(-nmax // 16) * 16)

    WqT16 = np.ascontiguousarray(Wq.T.astype(np.float16))          # (QD, AD)
    v_r = np.ascontiguousarray(v.reshape(DB, 128).T.astype(np.float16))
    sel16 = np.zeros((128, BLOC), dtype=np.float32)
    for b in range(BLOC):
        sel16[b * 16 : (b + 1) * 16, b] = 1.0
    sel16T = np.ascontiguousarray(sel16.T)

    in_maps = []
    for i in range(NCORES):
        sl = slice(i * BLOC, (i + 1) * BLOC)
        pmc = np.zeros((BLOC, DB, 128, P), dtype=np.float16)
        mko = np.zeros((BLOC, P), dtype=np.uint8)
        for b in range(BLOC):
            ix = idxs[i * BLOC + b]
            n = len(ix)
            pmc[b, :, :, :n] = (
                pm[i * BLOC + b, ix, :].astype(np.float16).T.reshape(DB, 128, n)
            )
            mko[b, :n] = 1
        q = query[sl]
        qT16 = np.ascontiguousarray(
            q.T.reshape(KB, 128, BLOC).transpose(1, 0, 2).reshape(128, KB * BLOC)
        ).astype(np.float16)
        in_maps.append(
            {
                "pm_c": pmc,
                "qT": qT16,
                "WqT": WqT16,
                "v_r": v_r,
                "mko": np.ascontiguousarray(mko),
                "sel16": sel16,
                "sel16T": sel16T,
            }
        )
    return P, in_maps, idxs


def run_spmd(P, in_maps, **kwargs):
    return run_bass_kernel_spmd(_get_nc(P), in_maps, list(range(NCORES)), **kwargs)


def scatter_out(res, idxs) -> np.ndarray:
    full = np.zeros((B, T), dtype=np.float32)
    for i in range(NCORES):
        o = res.results[i]["out"]
        for b in range(BLOC):
            ix = idxs[i * BLOC + b]
            full[i * BLOC + b, ix] = o[b, : len(ix)]
    return full


def kernel(query, processed_memory, mask, Wq, v) -> np.ndarray:
    P, in_maps, idxs = prep(query, processed_memory, mask, Wq, v)
    res = run_spmd(P, in_maps)
    return scatter_out(res, idxs)


# revision 6
# speedup vs baseline: 1.4650x; 1.0705x over previous
"""Bahdanau attention kernel for Trainium2 (8 NeuronCores, data-parallel over batch).

Reference computation (per batch row b):
    pq      = query @ Wq.T                       # (B, AD)
    hidden  = tanh(pq[:, None, :] + processed_memory)   # (B, T, AD)
    e       = einsum('btd,d->bt', hidden, v)     # (B, T)
    e       = where(mask, -1e30, e)
    out     = softmax(e, axis=1)

Key observation: ~50% of positions have mask=True, and for those the reference
output is *exactly* 0.0 (exp(-1e30 - max) underflows).  So the host gathers
only the unmasked positions per batch (n_b <= 2126 for this input family),
pads each batch to a common static length P, and the device only processes
the compacted stream - halving HBM traffic, tanh work and matmul work.  The
host scatters results back and fills masked positions with exact zeros.

Padding is self-masking: pad columns of pm are set to -30*sign(v_d), so
tanh(pq + pad) saturates to -sign(v_d) and the pad energy is exactly
-sum|v_d| ~= -12.8, giving exp(e_pad)/rowsum ~ 1e-9 - negligible in the
denominator, and the host scatter discards pad outputs anyway.  No mask
tensor ever reaches the device.

Device strategy (per core, 8 batches, compacted to [*, P]):
  * pm is host-gathered/transposed to [b, d-block, 128, P] fp16 so AD sits on
    SBUF partitions.  The "+pq" add folds into the ScalarE tanh as a
    per-partition activation bias, and the v-weighted reduction over d is
    TensorE matmuls with a [128,1] fp16 stationary v column.
  * Energy strips for a group of 4 batches are col-tiled to PSUM partitions
    0/32/64/96 of one [128, P] strip tile (tile_position=(0, 32j)), making
    the PSUM->SBUF evacuation a full-width DVE copy instead of a single-lane
    [1, P] copy.
  * Strips relay to a [128, P/16] softmax tile (16 partitions per batch) via
    HWDGE SBUF->SBUF DMAs; each group of 4 batches runs its own complete
    softmax (exp with fused accum_out row sums, selector matmuls for the
    cross-partition sum + broadcast, scale, store) as soon as it is ready, so
    group 0's output is already in HBM while group 1 still computes.
"""

import sys

if "/opt/trn_rl_repo" not in sys.path:
    sys.path.insert(0, "/opt/trn_rl_repo")

import numpy as np

import concourse.bacc as bacc
import concourse.bass as bass
import concourse.tile as tile
from concourse import mybir
from concourse.bass_utils import run_bass_kernel_spmd

B, T, QD, AD = 64, 4096, 1024, 256
NCORES = 8
BLOC = B // NCORES  # batches per core
KB = QD // 128      # k-blocks for the pq matmul
DB = AD // 128      # d-blocks (partition blocks of AD)
F32 = mybir.dt.float32
F16 = mybir.dt.float16

P_DEFAULT = 2176    # compacted positions per batch (multiple of 16, >= max n_b)


def build_nc(P: int) -> bass.Bass:
    assert P % 16 == 0 and P <= 2560
    PF = P // 16
    chunks = [(c, min(c + 512, P)) for c in range(0, P, 512)]

    nc = bacc.Bacc(None, target_bir_lowering=False)

    pm_c = nc.declare_dram_parameter("pm_c", [BLOC, DB, 128, P], F16, isOutput=False)
    qT = nc.declare_dram_parameter("qT", [128, KB * BLOC], F16, isOutput=False)
    WqT = nc.declare_dram_parameter("WqT", [QD, AD], F16, isOutput=False)
    v_r = nc.declare_dram_parameter("v_r", [128, DB], F16, isOutput=False)
    sel16_d = nc.declare_dram_parameter("sel16", [128, BLOC], F32, isOutput=False)
    selb_d = nc.declare_dram_parameter("selb", [4, 64], F32, isOutput=False)
    out = nc.declare_dram_parameter("out", [BLOC, P], F32, isOutput=True)
    out_r = out[:, :].rearrange("b (q f) -> (b q) f", f=PF)

    Tanh = mybir.ActivationFunctionType.Tanh
    Exp = mybir.ActivationFunctionType.Exp

    with tile.TileContext(nc) as tc:
        with (
            tc.tile_pool(name="singles", bufs=1) as singles,
            tc.tile_pool(name="pm", bufs=6) as pm_pool,
            tc.tile_pool(name="hid", bufs=6) as hid_pool,
            tc.tile_pool(name="es", bufs=2) as es_pool,
            tc.tile_pool(name="ps", bufs=1, space="PSUM") as ps_pool,
        ):
            # dummy tanh first: pulls the ACT_TABLE_LOAD off the critical path
            dummy = singles.tile([128, 1], F32)
            nc.gpsimd.memset(dummy, 0.0)
            dummy2 = singles.tile([128, 1], F32)
            nc.scalar.activation(out=dummy2, in_=dummy, func=Tanh)

            # rows not yet written by exp accum_out must be 0.0 (not garbage)
            # when the per-group row-sum matmul reads the full column
            colsum = singles.tile([128, 1], F32)
            nc.gpsimd.memset(colsum, 0.0)

            # ---- critical-path DMAs first on the sync queue:
            # wq/qt gate pq; pm b0/b1 gate the tanh stream ----
            wq_sb = singles.tile([128, KB, AD], F16)
            for db in range(DB):
                nc.sync.dma_start(
                    out=wq_sb[:, :, db * 128 : (db + 1) * 128],
                    in_=WqT[:, db * 128 : (db + 1) * 128].rearrange(
                        "(kb p) d -> p kb d", p=128
                    ),
                )
            qt_sb = singles.tile([128, KB, BLOC], F16)
            nc.sync.dma_start(
                out=qt_sb, in_=qT[:, :].rearrange("p (kb b) -> p kb b", b=BLOC)
            )
            pm_tiles = {}
            for b in range(2):
                for db in range(DB):
                    pm_sb = pm_pool.tile([128, P], F16, name=f"pm_{b}_{db}")
                    if b == 0:
                        h = P // 2
                        nc.sync.dma_start(
                            out=pm_sb[:, 0:h], in_=pm_c[b, db, :, 0:h]
                        )
                        nc.sync.dma_start(
                            out=pm_sb[:, h:P], in_=pm_c[b, db, :, h:P]
                        )
                    else:
                        nc.sync.dma_start(out=pm_sb, in_=pm_c[b, db, :, :])
                    pm_tiles[(b, db)] = pm_sb

            # small constants ride the gpsimd (SWDGE) queue
            v16 = singles.tile([128, DB], F16)
            nc.gpsimd.dma_start(out=v16, in_=v_r[:, :])
            sel16 = singles.tile([128, BLOC], F32)
            nc.gpsimd.dma_start(out=sel16, in_=sel16_d[:, :])
            selb = singles.tile([4, 64], F32)
            nc.gpsimd.dma_start(out=selb, in_=selb_d[:, :])

            # ---- pq = Wq @ query.T, laid out [d % 128, dblk, b] ----
            pq_sb = singles.tile([128, DB, BLOC], F32)
            for db in range(DB):
                ppq = ps_pool.tile([128, BLOC], F32, tag="pq", bufs=1)
                for k in range(KB):
                    nc.tensor.matmul(
                        ppq,
                        lhsT=wq_sb[:, k, db * 128 : (db + 1) * 128],
                        rhs=qt_sb[:, k, :],
                        start=(k == 0),
                        stop=(k == KB - 1),
                    )
                nc.vector.tensor_copy(out=pq_sb[:, db, :], in_=ppq)

            e2 = singles.tile([128, PF], F32)
            work2 = singles.tile([128, PF], F32)
            rinv = singles.tile([4, 1], F32)

            # ---- main loop ----
            ep = es = None
            for b in range(BLOC):
                g, j = b // 4, b % 4
                if j == 0:
                    ep = ps_pool.tile([128, P], F32, tag="strip", bufs=1)
                    es = es_pool.tile([128, P], F32)
                hids = []
                for db in range(DB):
                    if b < 2:
                        pm_sb = pm_tiles[(b, db)]
                    else:
                        pm_sb = pm_pool.tile([128, P], F16)
                        nc.sync.dma_start(out=pm_sb, in_=pm_c[b, db, :, :])
                    h = hid_pool.tile([128, P], F16)
                    if b == 0:
                        hp = P // 2
                        for c0, c1 in ((0, hp), (hp, P)):
                            nc.scalar.activation(
                                out=h[:, c0:c1],
                                in_=pm_sb[:, c0:c1],
                                func=Tanh,
                                bias=pq_sb[:, db, b : b + 1],
                                scale=1.0,
                            )
                    else:
                        nc.scalar.activation(
                            out=h,
                            in_=pm_sb,
                            func=Tanh,
                            bias=pq_sb[:, db, b : b + 1],
                            scale=1.0,
                        )
                    hids.append(h)
                row = ep[32 * j : 32 * j + 1, :]
                for c0, c1 in chunks:
                    nc.tensor.matmul(
                        row[:, c0:c1],
                        lhsT=v16[:, 0:1],
                        rhs=hids[0][:, c0:c1],
                        start=True,
                        stop=False,
                        tile_position=(0, 32 * j),
                    )
                for c0, c1 in chunks:
                    nc.tensor.matmul(
                        row[:, c0:c1],
                        lhsT=v16[:, 1:2],
                        rhs=hids[1][:, c0:c1],
                        start=False,
                        stop=True,
                        tile_position=(0, 32 * j),
                    )
                    if j == 3:
                        # evacuate each chunk as soon as its last matmul lands
                        nc.vector.tensor_copy(out=es[:, c0:c1], in_=ep[:, c0:c1])
                if j == 3:
                    # relayout the 4 strips into the [128, PF] softmax tile
                    for bb in range(g * 4, g * 4 + 4):
                        jj = bb % 4
                        nc.sync.dma_start(
                            out=e2[bb * 16 : (bb + 1) * 16, :],
                            in_=es[32 * jj : 32 * jj + 1, :],
                        )
            # group softmaxes: emitted after the NEXT group's first tanhs so
            # the ScalarE queue never stalls waiting on relayout DMAs; tile
            # deps alone order these correctly, emission slot only controls
            # the ScalarE FIFO position.  We emit group 0's chain here
            # (after the loop Tile will still schedule it early because its
            # deps resolve mid-stream) - see note below.
            for g in range(2):
                lo = g * 64
                nc.scalar.activation(
                    out=work2[lo : lo + 64, :],
                    in_=e2[lo : lo + 64, :],
                    func=Exp,
                    accum_out=colsum[lo : lo + 64, :],
                )
                psum_rs = ps_pool.tile([4, 1], F32, tag="red", bufs=1)
                nc.tensor.matmul(
                    psum_rs,
                    lhsT=sel16[:, 4 * g : 4 * g + 4],
                    rhs=colsum,
                    start=True,
                    stop=True,
                )
                nc.vector.reciprocal(out=rinv, in_=psum_rs)
                psum_ri = ps_pool.tile([128, 1], F32, tag="red", bufs=1)
                nc.tensor.matmul(
                    psum_ri[lo : lo + 64, :],
                    lhsT=selb,
                    rhs=rinv,
                    start=True,
                    stop=True,
                    tile_position=(0, 64 * g),
                )
                nc.vector.tensor_scalar_mul(
                    out=e2[lo : lo + 64, :],
                    in0=work2[lo : lo + 64, :],
                    scalar1=psum_ri[lo : lo + 64, :],
                )
                nc.sync.dma_start(
                    out=out_r[lo : lo + 64, :], in_=e2[lo : lo + 64, :]
                )

    nc.finalize()
    return nc


_CACHE: dict = {}


def _get_nc(P: int) -> bass.Bass:
    if P not in _CACHE:
        _CACHE[P] = build_nc(P)
    return _CACHE[P]


def prep(query, processed_memory, mask, Wq, v):
    """Host-side shard + compact.  Returns (P, in_maps, scatter_info)."""
    query = np.asarray(query, dtype=np.float32)
    pm = np.asarray(processed_memory, dtype=np.float32)
    mask_b = np.asarray(mask).astype(bool)
    Wq = np.asarray(Wq, dtype=np.float32)
    v = np.asarray(v, dtype=np.float32)

    idxs = [np.nonzero(~mask_b[b])[0] for b in range(B)]
    nmax = max((len(ix) for ix in idxs), default=0)
    P = max(P_DEFAULT, -(-nmax // 16) * 16)

    WqT16 = np.ascontiguousarray(Wq.T.astype(np.float16))          # (QD, AD)
    v_r = np.ascontiguousarray(v.reshape(DB, 128).T.astype(np.float16))
    # self-masking pad column: tanh(pq - 30*sign(v_d)) == -sign(v_d)
    pad_col = (-30.0 * np.sign(v).astype(np.float16)).reshape(DB, 128).T  # (128, DB)
    sel16 = np.zeros((128, BLOC), dtype=np.float32)
    for b in range(BLOC):
        sel16[b * 16 : (b + 1) * 16, b] = 1.0
    selb = np.zeros((4, 64), dtype=np.float32)
    for i in range(4):
        selb[i, i * 16 : (i + 1) * 16] = 1.0

    in_maps = []
    for i in range(NCORES):
        sl = slice(i * BLOC, (i + 1) * BLOC)
        pmc = np.empty((BLOC, DB, 128, P), dtype=np.float16)
        pmc[:] = pad_col.T[None, :, :, None]  # (DB,128) broadcast over b, t
        for b in range(BLOC):
            ix = idxs[i * BLOC + b]
            n = len(ix)
            pmc[b, :, :, :n] = (
                pm[i * BLOC + b, ix, :].astype(np.float16).T.reshape(DB, 128, n)
            )
        q = query[sl]
        qT16 = np.ascontiguousarray(
            q.T.reshape(KB, 128, BLOC).transpose(1, 0, 2).reshape(128, KB * BLOC)
        ).astype(np.float16)
        in_maps.append(
            {
                "pm_c": pmc,
                "qT": qT16,
                "WqT": WqT16,
                "v_r": v_r,
                "sel16": sel16,
                "selb": selb,
            }
        )
    return P, in_maps, idxs


def run_spmd(P, in_maps, **kwargs):
    return run_bass_kernel_spmd(_get_nc(P), in_maps, list(range(NCORES)), **kwargs)


def scatter_out(res, idxs) -> np.ndarray:
    full = np.zeros((B, T), dtype=np.float32)
    for i in range(NCORES):
        o = res.results[i]["out"]
        for b in range(BLOC):
            ix = idxs[i * BLOC + b]
            full[i * BLOC + b, ix] = o[b, : len(ix)]
    return full


def kernel(query, processed_memory, mask, Wq, v) -> np.ndarray:
    P, in_maps, idxs = prep(query, processed_memory, mask, Wq, v)
    res = run_spmd(P, in_maps)
    return scatter_out(res, idxs)


# revision 8
# speedup vs baseline: 1.5818x; 1.0797x over previous
"""Bahdanau attention kernel for Trainium2 (8 NeuronCores, data-parallel over batch).

Reference computation (per batch row b):
    pq      = query @ Wq.T                       # (B, AD)
    hidden  = tanh(pq[:, None, :] + processed_memory)   # (B, T, AD)
    e       = einsum('btd,d->bt', hidden, v)     # (B, T)
    e       = where(mask, -1e30, e)
    out     = softmax(e, axis=1)

Key observation: ~50% of positions have mask=True, and for those the reference
output is *exactly* 0.0 (exp(-1e30 - max) underflows).  So the host gathers
only the unmasked positions per batch (n_b <= 2126 for this input family),
pads each batch to a common static length P, and the device only processes
the compacted stream - halving HBM traffic, tanh work and matmul work.  The
host scatters results back and fills masked positions with exact zeros.

Padding is self-masking: pad columns of pm are set to -30*sign(v_d), so
tanh(pq + pad) saturates to -sign(v_d) and the pad energy is exactly
-sum|v_d| ~= -12.8, giving exp(e_pad)/rowsum ~ 1e-9 - negligible in the
denominator, and the host scatter discards pad outputs anyway.  No mask
tensor ever reaches the device.

Device strategy (per core, 8 batches, compacted to [*, P]):
  * pm is host-gathered/transposed to [b, d-block, 128, P] fp16 so AD sits on
    SBUF partitions.  The "+pq" add folds into the ScalarE tanh as a
    per-partition activation bias, and the v-weighted reduction over d is
    TensorE matmuls with a [128,1] fp16 stationary v column.
  * Energy strips for a group of 4 batches are col-tiled to PSUM partitions
    0/32/64/96 of one [128, P] strip tile (tile_position=(0, 32j)), making
    the PSUM->SBUF evacuation a full-width DVE copy instead of a single-lane
    [1, P] copy.
  * Strips relay to a [128, P/16] softmax tile (16 partitions per batch) via
    HWDGE SBUF->SBUF DMAs; each group of 4 batches runs its own complete
    softmax (exp with fused accum_out row sums, selector matmuls for the
    cross-partition sum + broadcast, scale, store) as soon as it is ready, so
    group 0's output is already in HBM while group 1 still computes.
"""

import sys

if "/opt/trn_rl_repo" not in sys.path:
    sys.path.insert(0, "/opt/trn_rl_repo")

import numpy as np

import concourse.bacc as bacc
import concourse.bass as bass
import concourse.tile as tile
from concourse import mybir
from concourse.bass_utils import run_bass_kernel_spmd

B, T, QD, AD = 64, 4096, 1024, 256
NCORES = 8
BLOC = B // NCORES  # batches per core
KB = QD // 128      # k-blocks for the pq matmul
DB = AD // 128      # d-blocks (partition blocks of AD)
F32 = mybir.dt.float32
F16 = mybir.dt.float16

P_DEFAULT = 2176    # compacted positions per batch (multiple of 16, >= max n_b)


def build_nc(P: int) -> bass.Bass:
    assert P % 16 == 0 and P <= 2560
    PF = P // 16
    chunks = [(c, min(c + 512, P)) for c in range(0, P, 512)]

    nc = bacc.Bacc(None, target_bir_lowering=False)

    pm_c = nc.declare_dram_parameter("pm_c", [BLOC, DB, 128, P], F16, isOutput=False)
    qT = nc.declare_dram_parameter("qT", [128, KB * BLOC], F16, isOutput=False)
    WqT = nc.declare_dram_parameter("WqT", [QD, AD], F16, isOutput=False)
    v_r = nc.declare_dram_parameter("v_r", [128, DB], F16, isOutput=False)
    sel16_d = nc.declare_dram_parameter("sel16", [128, BLOC], F32, isOutput=False)
    selb_d = nc.declare_dram_parameter("selb", [4, 64], F32, isOutput=False)
    out = nc.declare_dram_parameter("out", [BLOC, P], F32, isOutput=True)
    out_r = out[:, :].rearrange("b (q f) -> (b q) f", f=PF)

    Tanh = mybir.ActivationFunctionType.Tanh
    Exp = mybir.ActivationFunctionType.Exp

    with tile.TileContext(nc) as tc:
        with (
            tc.tile_pool(name="singles", bufs=1) as singles,
            tc.tile_pool(name="pm", bufs=6) as pm_pool,
            tc.tile_pool(name="hid", bufs=6) as hid_pool,
            tc.tile_pool(name="es", bufs=2) as es_pool,
            tc.tile_pool(name="ps", bufs=1, space="PSUM") as ps_pool,
        ):
            # dummy tanh first: pulls the ACT_TABLE_LOAD off the critical path
            dummy = singles.tile([128, 1], F32)
            nc.gpsimd.memset(dummy, 0.0)
            dummy2 = singles.tile([128, 1], F32)
            nc.scalar.activation(out=dummy2, in_=dummy, func=Tanh)

            # rows not yet written by exp accum_out must be 0.0 (not garbage)
            # when the per-group row-sum matmul reads the full column
            colsum = singles.tile([128, 1], F32)
            nc.gpsimd.memset(colsum, 0.0)

            # ---- critical-path DMAs first on the sync queue:
            # wq/qt gate pq; pm b0/b1 gate the tanh stream ----
            wq_sb = singles.tile([128, KB, AD], F16)
            for db in range(DB):
                nc.sync.dma_start(
                    out=wq_sb[:, :, db * 128 : (db + 1) * 128],
                    in_=WqT[:, db * 128 : (db + 1) * 128].rearrange(
                        "(kb p) d -> p kb d", p=128
                    ),
                )
            qt_sb = singles.tile([128, KB, BLOC], F16)
            nc.sync.dma_start(
                out=qt_sb, in_=qT[:, :].rearrange("p (kb b) -> p kb b", b=BLOC)
            )
            pm_tiles = {}
            for b in range(2):
                for db in range(DB):
                    pm_sb = pm_pool.tile([128, P], F16, name=f"pm_{b}_{db}")
                    if b == 0:
                        h = P // 2
                        nc.sync.dma_start(
                            out=pm_sb[:, 0:h], in_=pm_c[b, db, :, 0:h]
                        )
                        nc.sync.dma_start(
                            out=pm_sb[:, h:P], in_=pm_c[b, db, :, h:P]
                        )
                    else:
                        nc.sync.dma_start(out=pm_sb, in_=pm_c[b, db, :, :])
                    pm_tiles[(b, db)] = pm_sb

            # small constants ride the gpsimd (SWDGE) queue
            v16 = singles.tile([128, DB], F16)
            nc.gpsimd.dma_start(out=v16, in_=v_r[:, :])
            sel16 = singles.tile([128, BLOC], F32)
            nc.gpsimd.dma_start(out=sel16, in_=sel16_d[:, :])
            selb = singles.tile([4, 64], F32)
            nc.gpsimd.dma_start(out=selb, in_=selb_d[:, :])

            # ---- pq = Wq @ query.T, laid out [d % 128, dblk, b] ----
            pq_sb = singles.tile([128, DB, BLOC], F32)
            for db in range(DB):
                ppq = ps_pool.tile([128, BLOC], F32, tag="pq", bufs=1)
                for k in range(KB):
                    nc.tensor.matmul(
                        ppq,
                        lhsT=wq_sb[:, k, db * 128 : (db + 1) * 128],
                        rhs=qt_sb[:, k, :],
                        start=(k == 0),
                        stop=(k == KB - 1),
                    )
                nc.vector.tensor_copy(out=pq_sb[:, db, :], in_=ppq)

            e2 = singles.tile([128, PF], F32)
            work2 = singles.tile([128, PF], F32)
            rinv = singles.tile([4, 1], F32)

            # ---- main loop ----
            eps = es = None
            for b in range(BLOC):
                g, j = b // 4, b % 4
                if j == 0:
                    # one single-bank PSUM tile per 512-chunk: separate tiles
                    # keep the DVE evacuation copies from creating false
                    # WAR serialization against later matmuls
                    eps = [
                        ps_pool.tile(
                            [128, c1 - c0], F32, tag=f"c{ci}", bufs=1,
                            name=f"ep{g}_{ci}",
                        )
                        for ci, (c0, c1) in enumerate(chunks)
                    ]
                    es = es_pool.tile([128, P], F32)
                hids = []
                for db in range(DB):
                    if b < 2:
                        pm_sb = pm_tiles[(b, db)]
                    else:
                        pm_sb = pm_pool.tile([128, P], F16)
                        nc.sync.dma_start(out=pm_sb, in_=pm_c[b, db, :, :])
                    h = hid_pool.tile([128, P], F16)
                    if b == 0:
                        hp = P // 2
                        for c0, c1 in ((0, hp), (hp, P)):
                            nc.scalar.activation(
                                out=h[:, c0:c1],
                                in_=pm_sb[:, c0:c1],
                                func=Tanh,
                                bias=pq_sb[:, db, b : b + 1],
                                scale=1.0,
                            )
                    else:
                        nc.scalar.activation(
                            out=h,
                            in_=pm_sb,
                            func=Tanh,
                            bias=pq_sb[:, db, b : b + 1],
                            scale=1.0,
                        )
                    hids.append(h)
                for ci, (c0, c1) in enumerate(chunks):
                    nc.tensor.matmul(
                        eps[ci][32 * j : 32 * j + 1, :],
                        lhsT=v16[:, 0:1],
                        rhs=hids[0][:, c0:c1],
                        start=True,
                        stop=False,
                        tile_position=(0, 32 * j),
                    )
                for ci, (c0, c1) in enumerate(chunks):
                    nc.tensor.matmul(
                        eps[ci][32 * j : 32 * j + 1, :],
                        lhsT=v16[:, 1:2],
                        rhs=hids[1][:, c0:c1],
                        start=False,
                        stop=True,
                        tile_position=(0, 32 * j),
                    )
                    if j == 3:
                        # evacuate each chunk as soon as its last matmul lands
                        nc.vector.tensor_copy(out=es[:, c0:c1], in_=eps[ci])
                if j == 3:
                    # relayout all 4 strips into the [128, PF] softmax tile
                    # with one partition-strided DMA
                    nc.sync.dma_start(
                        out=e2[g * 64 : g * 64 + 64, :], in_=es[0:97:32, :]
                    )
            # group softmaxes: emitted after the NEXT group's first tanhs so
            # the ScalarE queue never stalls waiting on relayout DMAs; tile
            # deps alone order these correctly, emission slot only controls
            # the ScalarE FIFO position.  We emit group 0's chain here
            # (after the loop Tile will still schedule it early because its
            # deps resolve mid-stream) - see note below.
            for g in range(2):
                lo = g * 64
                nc.scalar.activation(
                    out=work2[lo : lo + 64, :],
                    in_=e2[lo : lo + 64, :],
                    func=Exp,
                    accum_out=colsum[lo : lo + 64, :],
                )
                psum_rs = ps_pool.tile([4, 1], F32, tag="red", bufs=1)
                nc.tensor.matmul(
                    psum_rs,
                    lhsT=sel16[:, 4 * g : 4 * g + 4],
                    rhs=colsum,
                    start=True,
                    stop=True,
                )
                nc.vector.reciprocal(out=rinv, in_=psum_rs)
                psum_ri = ps_pool.tile([128, 1], F32, tag="red", bufs=1)
                nc.tensor.matmul(
                    psum_ri[lo : lo + 64, :],
                    lhsT=selb,
                    rhs=rinv,
                    start=True,
                    stop=True,
                    tile_position=(0, 64 * g),
                )
                nc.vector.tensor_scalar_mul(
                    out=e2[lo : lo + 64, :],
                    in0=work2[lo : lo + 64, :],
                    scalar1=psum_ri[lo : lo + 64, :],
                )
                nc.sync.dma_start(
                    out=out_r[lo : lo + 64, :], in_=e2[lo : lo + 64, :]
                )

    nc.finalize()
    return nc


_CACHE: dict = {}


def _get_nc(P: int) -> bass.Bass:
    if P not in _CACHE:
        _CACHE[P] = build_nc(P)
    return _CACHE[P]


def prep(query, processed_memory, mask, Wq, v):
    """Host-side shard + compact.  Returns (P, in_maps, scatter_info)."""
    query = np.asarray(query, dtype=np.float32)
    pm = np.asarray(processed_memory, dtype=np.float32)
    mask_b = np.asarray(mask).astype(bool)
    Wq = np.asarray(Wq, dtype=np.float32)
    v = np.asarray(v, dtype=np.float32)

    idxs = [np.nonzero(~mask_b[b])[0] for b in range(B)]
    nmax = max((len(ix) for ix in idxs), default=0)
    P = max(P_DEFAULT, -(-nmax // 16) * 16)

    WqT16 = np.ascontiguousarray(Wq.T.astype(np.float16))          # (QD, AD)
    v_r = np.ascontiguousarray(v.reshape(DB, 128).T.astype(np.float16))
    # self-masking pad column: tanh(pq - 30*sign(v_d)) == -sign(v_d)
    pad_col = (-30.0 * np.sign(v).astype(np.float16)).reshape(DB, 128).T  # (128, DB)
    sel16 = np.zeros((128, BLOC), dtype=np.float32)
    for b in range(BLOC):
        sel16[b * 16 : (b + 1) * 16, b] = 1.0
    selb = np.zeros((4, 64), dtype=np.float32)
    for i in range(4):
        selb[i, i * 16 : (i + 1) * 16] = 1.0

    in_maps = []
    for i in range(NCORES):
        sl = slice(i * BLOC, (i + 1) * BLOC)
        pmc = np.empty((BLOC, DB, 128, P), dtype=np.float16)
        pmc[:] = pad_col.T[None, :, :, None]  # (DB,128) broadcast over b, t
        for b in range(BLOC):
            ix = idxs[i * BLOC + b]
            n = len(ix)
            pmc[b, :, :, :n] = (
                pm[i * BLOC + b, ix, :].astype(np.float16).T.reshape(DB, 128, n)
            )
        q = query[sl]
        qT16 = np.ascontiguousarray(
            q.T.reshape(KB, 128, BLOC).transpose(1, 0, 2).reshape(128, KB * BLOC)
        ).astype(np.float16)
        in_maps.append(
            {
                "pm_c": pmc,
                "qT": qT16,
                "WqT": WqT16,
                "v_r": v_r,
                "sel16": sel16,
                "selb": selb,
            }
        )
    return P, in_maps, idxs


def run_spmd(P, in_maps, **kwargs):
    return run_bass_kernel_spmd(_get_nc(P), in_maps, list(range(NCORES)), **kwargs)


def scatter_out(res, idxs) -> np.ndarray:
    full = np.zeros((B, T), dtype=np.float32)
    for i in range(NCORES):
        o = res.results[i]["out"]
        for b in range(BLOC):
            ix = idxs[i * BLOC + b]
            full[i * BLOC + b, ix] = o[b, : len(ix)]
    return full


def kernel(query, processed_memory, mask, Wq, v) -> np.ndarray:
    P, in_maps, idxs = prep(query, processed_memory, mask, Wq, v)
    res = run_spmd(P, in_maps)
    return scatter_out(res, idxs)


# revision 10
# speedup vs baseline: 1.6494x; 1.0427x over previous
"""Bahdanau attention kernel for Trainium2 (8 NeuronCores, data-parallel over batch).

Reference computation (per batch row b):
    pq      = query @ Wq.T                       # (B, AD)
    hidden  = tanh(pq[:, None, :] + processed_memory)   # (B, T, AD)
    e       = einsum('btd,d->bt', hidden, v)     # (B, T)
    e       = where(mask, -1e30, e)
    out     = softmax(e, axis=1)

Key observation: ~50% of positions have mask=True, and for those the reference
output is *exactly* 0.0 (exp(-1e30 - max) underflows).  So the host gathers
only the unmasked positions per batch (n_b <= ~2130 for this input family),
pads to a static per-group length, and the device only processes the
compacted stream - halving HBM traffic, tanh work and matmul work.  The host
scatters results back and fills masked positions with exact zeros.

Padding is self-masking: pad columns of pm are set to -30*sign(v_d), so
tanh(pq + pad) saturates to -sign(v_d) and the pad energy is exactly
-sum|v_d| ~= -12.8, giving exp(e_pad)/rowsum ~ 1e-9 - negligible in the
denominator; the host scatter discards pad outputs anyway.  No mask tensor
ever reaches the device.

Per-core batches are sorted by unmasked count: the 4 largest form group 0
(padded to P0), the 4 smallest group 1 (padded to P1 <= P0), trimming tanh /
matmul / DMA work on the second group.

Device strategy (per core, 8 batch slots):
  * pm is host-gathered/transposed to [slot, d-block, 128, P0] fp16 so AD
    sits on SBUF partitions; the "+pq" add folds into the ScalarE tanh as a
    per-partition bias; the v-weighted d-reduction is TensorE matmuls with a
    [128,1] fp16 stationary v column, col-tiled so the 4 slots of a group
    land on PSUM partitions 0/32/64/96 of shared single-bank chunk tiles.
  * Chunk tiles evacuate via full-width [128, 512] copies (VectorE, plus
    ScalarE for the tail group where ScalarE is otherwise idle), then one
    partition-strided SBUF->SBUF DMA relays each group into a [128, PF]
    softmax tile (16 rows per slot).
  * Per-group softmax: ScalarE exp with fused accum_out row sums, selector
    matmuls for the cross-partition sum + broadcast, one scale, one store -
    group 0's output is in HBM while group 1 still computes.
"""

import sys

if "/opt/trn_rl_repo" not in sys.path:
    sys.path.insert(0, "/opt/trn_rl_repo")

import numpy as np

import concourse.bacc as bacc
import concourse.bass as bass
import concourse.tile as tile
from concourse import mybir
from concourse.bass_utils import run_bass_kernel_spmd

B, T, QD, AD = 64, 4096, 1024, 256
NCORES = 8
BLOC = B // NCORES  # batch slots per core
KB = QD // 128      # k-blocks for the pq matmul
DB = AD // 128      # d-blocks (partition blocks of AD)
F32 = mybir.dt.float32
F16 = mybir.dt.float16

NCH = 5  # psum chunk tiles of 512 cols (supports P up to 2560)


def build_nc(P0: int, P1: int) -> bass.Bass:
    assert P0 % 16 == 0 and P1 % 16 == 0 and P1 <= P0 <= NCH * 512
    PF = [P0 // 16, P1 // 16]
    chunk_l = [
        [(c, min(c + 512, Pg)) for c in range(0, Pg, 512)] for Pg in (P0, P1)
    ]

    nc = bacc.Bacc(None, target_bir_lowering=False)

    pm_c = nc.declare_dram_parameter("pm_c", [BLOC, DB, 128, P0], F16, isOutput=False)
    qT = nc.declare_dram_parameter("qT", [128, KB * BLOC], F16, isOutput=False)
    WqT = nc.declare_dram_parameter("WqT", [QD, AD], F16, isOutput=False)
    v_r = nc.declare_dram_parameter("v_r", [128, DB], F16, isOutput=False)
    sel16_d = nc.declare_dram_parameter("sel16", [128, BLOC], F32, isOutput=False)
    selb_d = nc.declare_dram_parameter("selb", [4, 64], F32, isOutput=False)
    out = nc.declare_dram_parameter("out", [BLOC, P0], F32, isOutput=True)

    Tanh = mybir.ActivationFunctionType.Tanh
    Exp = mybir.ActivationFunctionType.Exp

    with tile.TileContext(nc) as tc:
        with (
            tc.tile_pool(name="singles", bufs=1) as singles,
            tc.tile_pool(name="pm", bufs=6) as pm_pool,
            tc.tile_pool(name="hid", bufs=6) as hid_pool,
            tc.tile_pool(name="es", bufs=2) as es_pool,
            tc.tile_pool(name="ps", bufs=1, space="PSUM") as ps_pool,
        ):
            # dummy tanh first: pulls the ACT_TABLE_LOAD off the critical path
            dummy = singles.tile([128, 1], F32)
            nc.gpsimd.memset(dummy, 0.0)
            dummy2 = singles.tile([128, 1], F32)
            nc.scalar.activation(out=dummy2, in_=dummy, func=Tanh)

            # rows not yet written by exp accum_out must be 0.0 (not garbage)
            # when the per-group row-sum matmul reads the full column
            colsum = singles.tile([128, 1], F32)
            nc.gpsimd.memset(colsum, 0.0)

            # ---- critical-path DMAs split across both queues:
            # sync: wq d-block 0, then the pm stream; gpsimd: wq d-block 1,
            # qT and the small constants ----
            wq_sb = singles.tile([128, KB, AD], F16)
            nc.sync.dma_start(
                out=wq_sb[:, :, 0:128],
                in_=WqT[:, 0:128].rearrange("(kb p) d -> p kb d", p=128),
            )
            nc.gpsimd.dma_start(
                out=wq_sb[:, :, 128:256],
                in_=WqT[:, 128:256].rearrange("(kb p) d -> p kb d", p=128),
            )
            qt_sb = singles.tile([128, KB, BLOC], F16)
            nc.gpsimd.dma_start(
                out=qt_sb, in_=qT[:, :].rearrange("p (kb b) -> p kb b", b=BLOC)
            )
            pm_tiles = {}
            for b in range(2):
                for db in range(DB):
                    pm_sb = pm_pool.tile([128, P0], F16, name=f"pm_{b}_{db}")
                    if b == 0:
                        h = P0 // 2
                        nc.sync.dma_start(
                            out=pm_sb[:, 0:h], in_=pm_c[b, db, :, 0:h]
                        )
                        nc.sync.dma_start(
                            out=pm_sb[:, h:P0], in_=pm_c[b, db, :, h:P0]
                        )
                    else:
                        nc.sync.dma_start(out=pm_sb, in_=pm_c[b, db, :, :])
                    pm_tiles[(b, db)] = pm_sb

            v16 = singles.tile([128, DB], F16)
            nc.gpsimd.dma_start(out=v16, in_=v_r[:, :])
            sel16 = singles.tile([128, BLOC], F32)
            nc.gpsimd.dma_start(out=sel16, in_=sel16_d[:, :])
            selb = singles.tile([4, 64], F32)
            nc.gpsimd.dma_start(out=selb, in_=selb_d[:, :])

            # ---- pq = Wq @ query.T, laid out [d % 128, dblk, b] ----
            pq_sb = singles.tile([128, DB, BLOC], F32)
            for db in range(DB):
                ppq = ps_pool.tile([128, BLOC], F32, tag="pq", bufs=1)
                for k in range(KB):
                    nc.tensor.matmul(
                        ppq,
                        lhsT=wq_sb[:, k, db * 128 : (db + 1) * 128],
                        rhs=qt_sb[:, k, :],
                        start=(k == 0),
                        stop=(k == KB - 1),
                    )
                nc.vector.tensor_copy(out=pq_sb[:, db, :], in_=ppq)

            e2 = singles.tile([128, PF[0]], F32)
            work2 = singles.tile([128, PF[0]], F32)
            rinv = singles.tile([4, 1], F32)

            # ---- main loop ----
            eps = es = None
            for b in range(BLOC):
                g, j = b // 4, b % 4
                Pg = (P0, P1)[g]
                chunks = chunk_l[g]
                if j == 0:
                    # one single-bank PSUM tile per 512-chunk: separate tiles
                    # keep the DVE evacuation copies from creating false WAR
                    # serialization against later matmuls
                    eps = [
                        ps_pool.tile(
                            [128, c1 - c0], F32, tag=f"c{ci}", bufs=1,
                            name=f"ep{g}_{ci}",
                        )
                        for ci, (c0, c1) in enumerate(chunks)
                    ]
                    es = es_pool.tile([128, Pg], F32, tag="es", name=f"es{g}")
                hids = []
                for db in range(DB):
                    if b < 2:
                        pm_sb = pm_tiles[(b, db)]
                    else:
                        pm_sb = pm_pool.tile([128, Pg], F16, tag="")
                        nc.sync.dma_start(out=pm_sb, in_=pm_c[b, db, :, 0:Pg])
                    h = hid_pool.tile([128, Pg], F16, tag="")
                    if b == 0:
                        hp = P0 // 2
                        for c0, c1 in ((0, hp), (hp, P0)):
                            nc.scalar.activation(
                                out=h[:, c0:c1],
                                in_=pm_sb[:, c0:c1],
                                func=Tanh,
                                bias=pq_sb[:, db, b : b + 1],
                                scale=1.0,
                            )
                    else:
                        nc.scalar.activation(
                            out=h,
                            in_=pm_sb[:, 0:Pg],
                            func=Tanh,
                            bias=pq_sb[:, db, b : b + 1],
                            scale=1.0,
                        )
                    hids.append(h)
                for ci, (c0, c1) in enumerate(chunks):
                    nc.tensor.matmul(
                        eps[ci][32 * j : 32 * j + 1, 0 : c1 - c0],
                        lhsT=v16[:, 0:1],
                        rhs=hids[0][:, c0:c1],
                        start=True,
                        stop=False,
                        tile_position=(0, 32 * j),
                    )
                for ci, (c0, c1) in enumerate(chunks):
                    nc.tensor.matmul(
                        eps[ci][32 * j : 32 * j + 1, 0 : c1 - c0],
                        lhsT=v16[:, 1:2],
                        rhs=hids[1][:, c0:c1],
                        start=False,
                        stop=True,
                        tile_position=(0, 32 * j),
                    )
                    if j == 3:
                        # evacuate each chunk as soon as its last matmul
                        # lands; for the tail group ScalarE is idle, so
                        # alternate engines to halve the copy stream
                        cp = eps[ci][:, 0 : c1 - c0]
                        if g == 1 and ci % 2 == 1:
                            nc.scalar.copy(es[:, c0:c1], cp)
                        else:
                            nc.vector.tensor_copy(out=es[:, c0:c1], in_=cp)
                if j == 3:
                    # relayout all 4 strips into the [128, PF] softmax tile
                    # with one partition-strided DMA
                    nc.sync.dma_start(
                        out=e2[g * 64 : g * 64 + 64, 0 : PF[g]],
                        in_=es[0:97:32, 0:Pg],
                    )
            # per-group softmax chains (deps gate execution; emission order
            # only sets engine FIFO position, after all tanhs)
            for g in range(2):
                lo = g * 64
                nc.scalar.activation(
                    out=work2[lo : lo + 64, 0 : PF[g]],
                    in_=e2[lo : lo + 64, 0 : PF[g]],
                    func=Exp,
                    accum_out=colsum[lo : lo + 64, :],
                )
                psum_rs = ps_pool.tile([4, 1], F32, tag="red", bufs=1)
                nc.tensor.matmul(
                    psum_rs,
                    lhsT=sel16[:, 4 * g : 4 * g + 4],
                    rhs=colsum,
                    start=True,
                    stop=True,
                )
                nc.vector.reciprocal(out=rinv, in_=psum_rs)
                psum_ri = ps_pool.tile([128, 1], F32, tag="red", bufs=1)
                nc.tensor.matmul(
                    psum_ri[lo : lo + 64, :],
                    lhsT=selb,
                    rhs=rinv,
                    start=True,
                    stop=True,
                    tile_position=(0, 64 * g),
                )
                nc.vector.tensor_scalar_mul(
                    out=e2[lo : lo + 64, 0 : PF[g]],
                    in0=work2[lo : lo + 64, 0 : PF[g]],
                    scalar1=psum_ri[lo : lo + 64, :],
                )
                nc.sync.dma_start(
                    out=out[4 * g : 4 * g + 4, 0 : (P0, P1)[g]].rearrange(
                        "b (q f) -> b q f", f=PF[g]
                    ),
                    in_=e2[lo : lo + 64, 0 : PF[g]],
                )

    nc.finalize()
    return nc


_CACHE: dict = {}


def _get_nc(P0: int, P1: int) -> bass.Bass:
    if (P0, P1) not in _CACHE:
        _CACHE[(P0, P1)] = build_nc(P0, P1)
    return _CACHE[(P0, P1)]


def _r16(n):
    return max(16, -(-n // 16) * 16)


def prep(query, processed_memory, mask, Wq, v):
    """Host-side shard + compact + sort.  Returns (P0, P1, in_maps, scatter)."""
    query = np.asarray(query, dtype=np.float32)
    pm = np.asarray(processed_memory, dtype=np.float32)
    mask_b = np.asarray(mask).astype(bool)
    Wq = np.asarray(Wq, dtype=np.float32)
    v = np.asarray(v, dtype=np.float32)

    idxs = [np.nonzero(~mask_b[b])[0] for b in range(B)]
    # per-core slot order: batches sorted by unmasked count, largest first;
    # slots 0-3 (group 0) pad to P0, slots 4-7 (group 1) to P1
    orders = []
    for i in range(NCORES):
        ns = [len(idxs[i * BLOC + b]) for b in range(BLOC)]
        orders.append(sorted(range(BLOC), key=lambda b: -ns[b]))
    P0 = _r16(max(len(idxs[i * BLOC + orders[i][0]]) for i in range(NCORES)))
    P1 = _r16(max(len(idxs[i * BLOC + orders[i][4]]) for i in range(NCORES)))
    P0 = max(P0, P1)

    WqT16 = np.ascontiguousarray(Wq.T.astype(np.float16))          # (QD, AD)
    v_r = np.ascontiguousarray(v.reshape(DB, 128).T.astype(np.float16))
    # self-masking pad column: tanh(pq - 30*sign(v_d)) == -sign(v_d)
    pad_col = (-30.0 * np.sign(v).astype(np.float16)).reshape(DB, 128)
    sel16 = np.zeros((128, BLOC), dtype=np.float32)
    for b in range(BLOC):
        sel16[b * 16 : (b + 1) * 16, b] = 1.0
    selb = np.zeros((4, 64), dtype=np.float32)
    for i in range(4):
        selb[i, i * 16 : (i + 1) * 16] = 1.0

    in_maps = []
    for i in range(NCORES):
        pmc = np.empty((BLOC, DB, 128, P0), dtype=np.float16)
        pmc[:] = pad_col[None, :, :, None]
        q_sl = np.empty((BLOC, QD), dtype=np.float32)
        for s in range(BLOC):
            bg = i * BLOC + orders[i][s]
            ix = idxs[bg]
            n = len(ix)
            pmc[s, :, :, :n] = pm[bg, ix, :].astype(np.float16).T.reshape(DB, 128, n)
            q_sl[s] = query[bg]
        qT16 = np.ascontiguousarray(
            q_sl.T.reshape(KB, 128, BLOC).transpose(1, 0, 2).reshape(128, KB * BLOC)
        ).astype(np.float16)
        in_maps.append(
            {
                "pm_c": pmc,
                "qT": qT16,
                "WqT": WqT16,
                "v_r": v_r,
                "sel16": sel16,
                "selb": selb,
            }
        )
    return P0, P1, in_maps, (idxs, orders)


def run_spmd(P0, P1, in_maps, **kwargs):
    return run_bass_kernel_spmd(
        _get_nc(P0, P1), in_maps, list(range(NCORES)), **kwargs
    )


def scatter_out(res, scatter) -> np.ndarray:
    idxs, orders = scatter
    full = np.zeros((B, T), dtype=np.float32)
    for i in range(NCORES):
        o = res.results[i]["out"]
        for s in range(BLOC):
            bg = i * BLOC + orders[i][s]
            ix = idxs[bg]
            full[bg, ix] = o[s, : len(ix)]
    return full


def kernel(query, processed_memory, mask, Wq, v) -> np.ndarray:
    P0, P1, in_maps, scatter = prep(query, processed_memory, mask, Wq, v)
    res = run_spmd(P0, P1, in_maps)
    return scatter_out(res, scatter)


# revision 13
# speedup vs baseline: 1.6670x; 1.0107x over previous
"""Bahdanau attention kernel for Trainium2 (8 NeuronCores, data-parallel over batch).

Reference computation (per batch row b):
    pq      = query @ Wq.T                       # (B, AD)
    hidden  = tanh(pq[:, None, :] + processed_memory)   # (B, T, AD)
    e       = einsum('btd,d->bt', hidden, v)     # (B, T)
    e       = where(mask, -1e30, e)
    out     = softmax(e, axis=1)

Key observation: ~50% of positions have mask=True, and for those the reference
output is *exactly* 0.0 (exp(-1e30 - max) underflows).  So the host gathers
only the unmasked positions per batch (n_b <= ~2130 for this input family),
pads to a static per-group length, and the device only processes the
compacted stream - halving HBM traffic, tanh work and matmul work.  The host
scatters results back and fills masked positions with exact zeros.

Padding is self-masking: pad columns of pm are set to -30*sign(v_d), so
tanh(pq + pad) saturates to -sign(v_d) and the pad energy is exactly
-sum|v_d| ~= -12.8, giving exp(e_pad)/rowsum ~ 1e-9 - negligible in the
denominator; the host scatter discards pad outputs anyway.  No mask tensor
ever reaches the device.

Per-core batches are sorted by unmasked count: the 4 largest form group 0
(padded to P0), the 4 smallest group 1 (padded to P1 <= P0), trimming tanh /
matmul / DMA work on the second group.

Device strategy (per core, 8 batch slots):
  * pm is host-gathered/transposed to [slot, d-block, 128, P0] fp16 so AD
    sits on SBUF partitions; the "+pq" add folds into the ScalarE tanh as a
    per-partition bias; the v-weighted d-reduction is TensorE matmuls with a
    [128,1] fp16 stationary v column, col-tiled so the 4 slots of a group
    land on PSUM partitions 0/32/64/96 of shared single-bank chunk tiles.
  * Chunk tiles evacuate via full-width [128, 512] copies (VectorE, plus
    ScalarE for the tail group where ScalarE is otherwise idle), then one
    partition-strided SBUF->SBUF DMA relays each group into a [128, PF]
    softmax tile (16 rows per slot).
  * Per-group softmax: ScalarE exp with fused accum_out row sums, selector
    matmuls for the cross-partition sum + broadcast, one scale, one store -
    group 0's output is in HBM while group 1 still computes.
"""

import sys

if "/opt/trn_rl_repo" not in sys.path:
    sys.path.insert(0, "/opt/trn_rl_repo")

import numpy as np

import concourse.bacc as bacc
import concourse.bass as bass
import concourse.tile as tile
from concourse import mybir
from concourse.bass_utils import run_bass_kernel_spmd

B, T, QD, AD = 64, 4096, 1024, 256
NCORES = 8
BLOC = B // NCORES  # batch slots per core
KB = QD // 128      # k-blocks for the pq matmul
DB = AD // 128      # d-blocks (partition blocks of AD)
F32 = mybir.dt.float32
F16 = mybir.dt.float16

NCH = 5  # psum chunk tiles of 512 cols (supports P up to 2560)


def build_nc(P0: int, P1: int) -> bass.Bass:
    assert P0 % 16 == 0 and P1 % 16 == 0 and P1 <= P0 <= NCH * 512
    PF = [P0 // 16, P1 // 16]
    chunk_l = [
        [(c, min(c + 512, Pg)) for c in range(0, Pg, 512)] for Pg in (P0, P1)
    ]

    nc = bacc.Bacc(None, target_bir_lowering=False)

    pm_c = nc.declare_dram_parameter("pm_c", [BLOC, DB, 128, P0], F16, isOutput=False)
    qT = nc.declare_dram_parameter("qT", [128, KB * BLOC], F16, isOutput=False)
    WqT = nc.declare_dram_parameter("WqT", [QD, AD], F16, isOutput=False)
    v_r = nc.declare_dram_parameter("v_r", [128, DB], F16, isOutput=False)
    sel16_d = nc.declare_dram_parameter("sel16", [128, BLOC], F16, isOutput=False)
    selb_d = nc.declare_dram_parameter("selb", [4, 64], F16, isOutput=False)
    out = nc.declare_dram_parameter("out", [BLOC, P0], F32, isOutput=True)

    Tanh = mybir.ActivationFunctionType.Tanh
    Exp = mybir.ActivationFunctionType.Exp

    with tile.TileContext(nc) as tc:
        with (
            tc.tile_pool(name="singles", bufs=1) as singles,
            tc.tile_pool(name="pm", bufs=6) as pm_pool,
            tc.tile_pool(name="hid", bufs=6) as hid_pool,
            tc.tile_pool(name="es", bufs=2) as es_pool,
            tc.tile_pool(name="ps", bufs=1, space="PSUM") as ps_pool,
        ):
            # dummy tanh first: pulls the ACT_TABLE_LOAD off the critical path
            dummy = singles.tile([128, 1], F32)
            nc.gpsimd.memset(dummy, 0.0)
            dummy2 = singles.tile([128, 1], F32)
            nc.scalar.activation(out=dummy2, in_=dummy, func=Tanh)

            # rows not yet written by exp accum_out must be 0.0 (not garbage)
            # when the per-group row-sum matmul reads the full column
            colsum = singles.tile([128, 1], F32)
            nc.gpsimd.memset(colsum, 0.0)

            # ---- critical-path DMAs split across both queues:
            # sync: wq d-block 0, then the pm stream; gpsimd: wq d-block 1,
            # qT and the small constants ----
            qt_sb = singles.tile([128, KB, BLOC], F16)
            nc.sync.dma_start(
                out=qt_sb, in_=qT[:, :].rearrange("p (kb b) -> p kb b", b=BLOC)
            )
            wq_sb = singles.tile([128, KB, AD], F16)
            for ka in range(2):
                nc.sync.dma_start(
                    out=wq_sb[:, 4 * ka : 4 * ka + 4, 0:128],
                    in_=WqT[512 * ka : 512 * ka + 512, 0:128].rearrange(
                        "(kb p) d -> p kb d", p=128
                    ),
                )
            nc.gpsimd.dma_start(
                out=wq_sb[:, :, 128:256],
                in_=WqT[:, 128:256].rearrange("(kb p) d -> p kb d", p=128),
            )
            pm_tiles = {}
            for b in range(2):
                for db in range(DB):
                    pm_sb = pm_pool.tile([128, P0], F16, name=f"pm_{b}_{db}")
                    if b == 0:
                        h = P0 // 2
                        nc.sync.dma_start(
                            out=pm_sb[:, 0:h], in_=pm_c[b, db, :, 0:h]
                        )
                        nc.sync.dma_start(
                            out=pm_sb[:, h:P0], in_=pm_c[b, db, :, h:P0]
                        )
                    else:
                        nc.sync.dma_start(out=pm_sb, in_=pm_c[b, db, :, :])
                    pm_tiles[(b, db)] = pm_sb

            v16 = singles.tile([128, DB], F16)
            nc.gpsimd.dma_start(out=v16, in_=v_r[:, :])
            sel16 = singles.tile([128, BLOC], F16)
            nc.gpsimd.dma_start(out=sel16, in_=sel16_d[:, :])
            selb = singles.tile([4, 64], F16)
            nc.gpsimd.dma_start(out=selb, in_=selb_d[:, :])

            # ---- pq = Wq @ query.T, laid out [d % 128, dblk, b] ----
            pq_sb = singles.tile([128, DB, BLOC], F32)
            for db in range(DB):
                ppq = ps_pool.tile([128, BLOC], F32, tag="pq", bufs=1)
                for k in range(KB):
                    nc.tensor.matmul(
                        ppq,
                        lhsT=wq_sb[:, k, db * 128 : (db + 1) * 128],
                        rhs=qt_sb[:, k, :],
                        start=(k == 0),
                        stop=(k == KB - 1),
                    )
                nc.vector.tensor_copy(out=pq_sb[:, db, :], in_=ppq)

            e2 = singles.tile([128, PF[0]], F32)
            work2 = singles.tile([128, PF[0]], F32)
            rinv = singles.tile([4, 1], F16)
            colsum16 = singles.tile([128, 1], F16)

            # ---- main loop ----
            eps = es = None
            for b in range(BLOC):
                g, j = b // 4, b % 4
                Pg = (P0, P1)[g]
                chunks = chunk_l[g]
                if j == 0:
                    # one single-bank PSUM tile per 512-chunk: separate tiles
                    # keep the DVE evacuation copies from creating false WAR
                    # serialization against later matmuls
                    eps = [
                        ps_pool.tile(
                            [128, c1 - c0], F32, tag=f"c{ci}", bufs=1,
                            name=f"ep{g}_{ci}",
                        )
                        for ci, (c0, c1) in enumerate(chunks)
                    ]
                    es = es_pool.tile([128, Pg], F32, tag="es", name=f"es{g}")
                hids = []
                for db in range(DB):
                    if b < 2:
                        pm_sb = pm_tiles[(b, db)]
                    else:
                        pm_sb = pm_pool.tile([128, Pg], F16, tag="")
                        nc.sync.dma_start(out=pm_sb, in_=pm_c[b, db, :, 0:Pg])
                    h = hid_pool.tile([128, Pg], F16, tag="")
                    if b == 0:
                        hp = P0 // 2
                        for c0, c1 in ((0, hp), (hp, P0)):
                            nc.scalar.activation(
                                out=h[:, c0:c1],
                                in_=pm_sb[:, c0:c1],
                                func=Tanh,
                                bias=pq_sb[:, db, b : b + 1],
                                scale=1.0,
                            )
                    else:
                        nc.scalar.activation(
                            out=h,
                            in_=pm_sb[:, 0:Pg],
                            func=Tanh,
                            bias=pq_sb[:, db, b : b + 1],
                            scale=1.0,
                        )
                    hids.append(h)
                for ci, (c0, c1) in enumerate(chunks):
                    nc.tensor.matmul(
                        eps[ci][32 * j : 32 * j + 1, 0 : c1 - c0],
                        lhsT=v16[:, 0:1],
                        rhs=hids[0][:, c0:c1],
                        start=True,
                        stop=False,
                        tile_position=(0, 32 * j),
                    )
                for ci, (c0, c1) in enumerate(chunks):
                    nc.tensor.matmul(
                        eps[ci][32 * j : 32 * j + 1, 0 : c1 - c0],
                        lhsT=v16[:, 1:2],
                        rhs=hids[1][:, c0:c1],
                        start=False,
                        stop=True,
                        tile_position=(0, 32 * j),
                    )
                    if j == 3:
                        # evacuate each chunk as soon as its last matmul
                        # lands; for the tail group ScalarE is idle, so
                        # alternate engines to halve the copy stream
                        cp = eps[ci][:, 0 : c1 - c0]
                        if g == 1 and ci % 2 == 1:
                            nc.scalar.copy(es[:, c0:c1], cp)
                        else:
                            nc.vector.tensor_copy(out=es[:, c0:c1], in_=cp)
                if j == 3:
                    # relayout all 4 strips into the [128, PF] softmax tile
                    # with one partition-strided DMA
                    nc.sync.dma_start(
                        out=e2[g * 64 : g * 64 + 64, 0 : PF[g]],
                        in_=es[0:97:32, 0:Pg],
                    )
            # per-group softmax chains (deps gate execution; emission order
            # only sets engine FIFO position, after all tanhs)
            for g in range(2):
                lo = g * 64
                nc.scalar.activation(
                    out=work2[lo : lo + 64, 0 : PF[g]],
                    in_=e2[lo : lo + 64, 0 : PF[g]],
                    func=Exp,
                    accum_out=colsum[lo : lo + 64, :],
                )
                with nc.allow_low_precision(reason="fp16 rowsum: 5e-4 err vs 2e-2 budget"):
                    nc.vector.tensor_copy(out=colsum16, in_=colsum)
                psum_rs = ps_pool.tile([4, 1], F32, tag="red", bufs=1)
                nc.tensor.matmul(
                    psum_rs,
                    lhsT=sel16[:, 4 * g : 4 * g + 4],
                    rhs=colsum16,
                    start=True,
                    stop=True,
                )
                with nc.allow_low_precision(reason="fp16 rowsum: 5e-4 err vs 2e-2 budget"):
                    nc.vector.reciprocal(out=rinv, in_=psum_rs)
                psum_ri = ps_pool.tile([128, 1], F32, tag="red", bufs=1)
                nc.tensor.matmul(
                    psum_ri[lo : lo + 64, :],
                    lhsT=selb,
                    rhs=rinv,
                    start=True,
                    stop=True,
                    tile_position=(0, 64 * g),
                )
                nc.vector.tensor_scalar_mul(
                    out=e2[lo : lo + 64, 0 : PF[g]],
                    in0=work2[lo : lo + 64, 0 : PF[g]],
                    scalar1=psum_ri[lo : lo + 64, :],
                )
                nc.sync.dma_start(
                    out=out[4 * g : 4 * g + 4, 0 : (P0, P1)[g]].rearrange(
                        "b (q f) -> b q f", f=PF[g]
                    ),
                    in_=e2[lo : lo + 64, 0 : PF[g]],
                )

    nc.finalize()
    return nc


_CACHE: dict = {}


def _get_nc(P0: int, P1: int) -> bass.Bass:
    if (P0, P1) not in _CACHE:
        _CACHE[(P0, P1)] = build_nc(P0, P1)
    return _CACHE[(P0, P1)]


def _r16(n):
    return max(16, -(-n // 16) * 16)


def prep(query, processed_memory, mask, Wq, v):
    """Host-side shard + compact + sort.  Returns (P0, P1, in_maps, scatter)."""
    query = np.asarray(query, dtype=np.float32)
    pm = np.asarray(processed_memory, dtype=np.float32)
    mask_b = np.asarray(mask).astype(bool)
    Wq = np.asarray(Wq, dtype=np.float32)
    v = np.asarray(v, dtype=np.float32)

    idxs = [np.nonzero(~mask_b[b])[0] for b in range(B)]
    # per-core slot order: batches sorted by unmasked count, largest first;
    # slots 0-3 (group 0) pad to P0, slots 4-7 (group 1) to P1
    orders = []
    for i in range(NCORES):
        ns = [len(idxs[i * BLOC + b]) for b in range(BLOC)]
        orders.append(sorted(range(BLOC), key=lambda b: -ns[b]))
    P0 = _r16(max(len(idxs[i * BLOC + orders[i][0]]) for i in range(NCORES)))
    P1 = _r16(max(len(idxs[i * BLOC + orders[i][4]]) for i in range(NCORES)))
    P0 = max(P0, P1)

    WqT16 = np.ascontiguousarray(Wq.T.astype(np.float16))          # (QD, AD)
    v_r = np.ascontiguousarray(v.reshape(DB, 128).T.astype(np.float16))
    # self-masking pad column: tanh(pq - 30*sign(v_d)) == -sign(v_d)
    pad_col = (-30.0 * np.sign(v).astype(np.float16)).reshape(DB, 128)
    sel16 = np.zeros((128, BLOC), dtype=np.float16)
    for b in range(BLOC):
        sel16[b * 16 : (b + 1) * 16, b] = 1.0
    selb = np.zeros((4, 64), dtype=np.float16)
    for i in range(4):
        selb[i, i * 16 : (i + 1) * 16] = 1.0

    in_maps = []
    for i in range(NCORES):
        pmc = np.empty((BLOC, DB, 128, P0), dtype=np.float16)
        pmc[:] = pad_col[None, :, :, None]
        q_sl = np.empty((BLOC, QD), dtype=np.float32)
        for s in range(BLOC):
            bg = i * BLOC + orders[i][s]
            ix = idxs[bg]
            n = len(ix)
            pmc[s, :, :, :n] = pm[bg, ix, :].astype(np.float16).T.reshape(DB, 128, n)
            q_sl[s] = query[bg]
        qT16 = np.ascontiguousarray(
            q_sl.T.reshape(KB, 128, BLOC).transpose(1, 0, 2).reshape(128, KB * BLOC)
        ).astype(np.float16)
        in_maps.append(
            {
                "pm_c": pmc,
                "qT": qT16,
                "WqT": WqT16,
                "v_r": v_r,
                "sel16": sel16,
                "selb": selb,
            }
        )
    return P0, P1, in_maps, (idxs, orders)


def run_spmd(P0, P1, in_maps, **kwargs):
    return run_bass_kernel_spmd(
        _get_nc(P0, P1), in_maps, list(range(NCORES)), **kwargs
    )


def scatter_out(res, scatter) -> np.ndarray:
    idxs, orders = scatter
    full = np.zeros((B, T), dtype=np.float32)
    for i in range(NCORES):
        o = res.results[i]["out"]
        for s in range(BLOC):
            bg = i * BLOC + orders[i][s]
            ix = idxs[bg]
            full[bg, ix] = o[s, : len(ix)]
    return full


def kernel(query, processed_memory, mask, Wq, v) -> np.ndarray:
    P0, P1, in_maps, scatter = prep(query, processed_memory, mask, Wq, v)
    res = run_spmd(P0, P1, in_maps)
    return scatter_out(res, scatter)
